# revision 4
# baseline (speedup 1.0000x reference)
"""Causal attention (B=4,T=4096,Dm=1024,Dk=256) on 8 TRN2 NeuronCores.

Sharding: 8 cores = 4 batches x 2 query-groups. Query dim split into eight
512-wide chunks; group h=0 takes chunks (7,5,2,0), h=1 takes (6,4,3,1) so
that causal work balances. Each core runs an IDENTICAL graph with 4 query
slots of uniform key-tile extents (32,24,16,8); the causal mask (incl. the
padding beyond a chunk's real causal extent) is pure data: a per-(slot,
key-tile) threshold column and `(col_iota >= thr) * exp(S/16)` fused on DVE.

Everything on-chip is computed in the transposed layout (Q^T,K^T,S^T,O^T)
so no PE transposes are needed; softmax is unnormalized (no max subtraction
-- scores/16 are O(1)) and the division by l plus the final transpose is
done on host. Compute dtype bf16, accumulation fp32.
"""

import math
import numpy as np
from contextlib import ExitStack

B, T, DM, DK = 4, 4096, 1024, 256
QW = 512                      # query chunk width
NSLOT = 4                     # query slots per core
SLOT_EXT = (32, 24, 16, 8)    # key-tile extent per slot (uniform across cores)
CHUNKS_H = {0: (7, 5, 2, 0), 1: (6, 4, 3, 1)}  # chunk idx per slot, per group
NKT = T // 128                # 32 key tiles
SCALE = 1.0 / math.sqrt(DK)   # 1/16

_CACHE = {}


def _build_graph():
    from concourse import bacc, mybir, tile

    f32 = mybir.dt.float32
    bf16 = mybir.dt.bfloat16
    AL = mybir.AluOpType

    nc = bacc.Bacc(None, target_bir_lowering=False)
    xt = nc.declare_dram_parameter("xt", [DM, T], bf16, isOutput=False)
    xq = nc.declare_dram_parameter("xq", [DM, NSLOT * QW], bf16, isOutput=False)
    wq = nc.declare_dram_parameter("wq", [DM, DK], bf16, isOutput=False)
    wk = nc.declare_dram_parameter("wk", [DM, DK], bf16, isOutput=False)
    wv = nc.declare_dram_parameter("wv", [DM, DK], bf16, isOutput=False)
    bqk = nc.declare_dram_parameter("bqk", [128, 4], f32, isOutput=False)
    bvb = nc.declare_dram_parameter("bvb", [128, DK], f32, isOutput=False)
    colio = nc.declare_dram_parameter("colio", [128, QW], f32, isOutput=False)
    thr = nc.declare_dram_parameter("thr", [128, NSLOT * NKT], f32, isOutput=False)
    ones = nc.declare_dram_parameter("ones", [128, 1], bf16, isOutput=False)
    o_t = nc.declare_dram_parameter("o_t", [NSLOT, DK, QW], f32, isOutput=True)
    l_o = nc.declare_dram_parameter("l_o", [NSLOT, 1, QW], f32, isOutput=True)

    with tile.TileContext(nc) as tc, ExitStack() as ctx:
        const = ctx.enter_context(tc.tile_pool(name="const", bufs=1))
        xt_pool = ctx.enter_context(tc.tile_pool(name="xt_pool", bufs=8))
        xq_pool = ctx.enter_context(tc.tile_pool(name="xq_pool", bufs=8))
        kvq = ctx.enter_context(tc.tile_pool(name="kvq", bufs=1))
        p_pool = ctx.enter_context(tc.tile_pool(name="p_pool", bufs=4))
        ps_pool = ctx.enter_context(
            tc.tile_pool(name="ps_pool", bufs=2, space="PSUM"))
        o_pool = ctx.enter_context(
            tc.tile_pool(name="o_pool", bufs=2, space="PSUM"))
        l_pool = ctx.enter_context(
            tc.tile_pool(name="l_pool", bufs=2, space="PSUM"))

        # constants / weights
        wq_sb = const.tile([128, 8, DK], bf16, tag="wq")
        wk_sb = const.tile([128, 8, DK], bf16, tag="wk")
        wv_sb = const.tile([128, 8, DK], bf16, tag="wv")
        nc.sync.dma_start(wq_sb[:], wq.rearrange("(c p) n -> p c n", p=128))
        nc.sync.dma_start(wk_sb[:], wk.rearrange("(c p) n -> p c n", p=128))
        nc.sync.dma_start(wv_sb[:], wv.rearrange("(c p) n -> p c n", p=128))
        bqk_sb = const.tile([128, 4], f32, tag="bqk")
        bvb_sb = const.tile([128, DK], f32, tag="bvb")
        colio_sb = const.tile([128, QW], f32, tag="colio")
        thr_sb = const.tile([128, NSLOT * NKT], f32, tag="thr")
        ones_sb = const.tile([128, 1], bf16, tag="ones")
        nc.sync.dma_start(bqk_sb[:], bqk[:])
        nc.sync.dma_start(bvb_sb[:], bvb[:])
        nc.sync.dma_start(colio_sb[:], colio[:])
        nc.sync.dma_start(thr_sb[:], thr[:])
        nc.sync.dma_start(ones_sb[:], ones[:])

        # activations
        xt_c = []
        for c in range(8):
            t_ = xt_pool.tile([128, T], bf16, tag="xt")
            nc.sync.dma_start(t_[:], xt[128 * c:128 * (c + 1), :])
            xt_c.append(t_)
        xq_c = []
        for c in range(8):
            t_ = xq_pool.tile([128, NSLOT * QW], bf16, tag="xq")
            nc.sync.dma_start(t_[:], xq[128 * c:128 * (c + 1), :])
            xq_c.append(t_)

        kt_sb = kvq.tile([128, 2, T], bf16, tag="kt")       # K^T
        vt_sb = kvq.tile([128, NKT, DK], bf16, tag="vt")    # V  (Ts-part)
        qt_sb = kvq.tile([128, 2, NSLOT * QW], bf16, tag="qt")  # Q^T

        # ---- projections ----
        # Q^T[dk, t] = sum_c Wq[c,dk]^T @ Xq[c,t]  (+bq per-partition)
        for dkc in range(2):
            for tj in range(NSLOT):
                ps = ps_pool.tile([128, QW], f32, tag="ps")
                for c in range(8):
                    nc.tensor.matmul(
                        ps[:],
                        wq_sb[:, c, 128 * dkc:128 * (dkc + 1)],
                        xq_c[c][:, QW * tj:QW * (tj + 1)],
                        start=(c == 0), stop=(c == 7))
                nc.vector.tensor_scalar(
                    qt_sb[:, dkc, QW * tj:QW * (tj + 1)], ps[:],
                    bqk_sb[:, dkc:dkc + 1], None, AL.add)
        # K^T
        for dkc in range(2):
            for tj in range(T // QW):
                ps = ps_pool.tile([128, QW], f32, tag="ps")
                for c in range(8):
                    nc.tensor.matmul(
                        ps[:],
                        wk_sb[:, c, 128 * dkc:128 * (dkc + 1)],
                        xt_c[c][:, QW * tj:QW * (tj + 1)],
                        start=(c == 0), stop=(c == 7))
                nc.vector.tensor_scalar(
                    kt_sb[:, dkc, QW * tj:QW * (tj + 1)], ps[:],
                    bqk_sb[:, 2 + dkc:3 + dkc], None, AL.add)
        # V natural layout: V[ts, dv]
        for tt in range(NKT):
            ps = ps_pool.tile([128, QW], f32, tag="ps")
            for c in range(8):
                nc.tensor.matmul(
                    ps[:, :DK],
                    xt_c[c][:, 128 * tt:128 * (tt + 1)],
                    wv_sb[:, c, :],
                    start=(c == 0), stop=(c == 7))
            nc.vector.tensor_tensor(
                vt_sb[:, tt, :], ps[:, :DK], bvb_sb[:], AL.add)

        # ---- attention (transposed layout) ----
        for j in range(NSLOT):
            E = SLOT_EXT[j]
            o_ps = o_pool.tile([128, 2, QW], f32, tag="o")
            l_ps = l_pool.tile([1, QW], f32, tag="l")
            for kt in range(E):
                s_ps = ps_pool.tile([128, QW], f32, tag="ps")
                for dkc in range(2):
                    nc.tensor.matmul(
                        s_ps[:],
                        kt_sb[:, dkc, 128 * kt:128 * (kt + 1)],
                        qt_sb[:, dkc, QW * j:QW * (j + 1)],
                        start=(dkc == 0), stop=(dkc == 1))
                p_raw = p_pool.tile([128, QW], bf16, tag="praw")
                nc.scalar.activation(
                    p_raw[:], s_ps[:],
                    mybir.ActivationFunctionType.Exp, scale=SCALE)
                p_m = p_pool.tile([128, QW], bf16, tag="pm")
                nc.vector.scalar_tensor_tensor(
                    p_m[:], colio_sb[:], thr_sb[:, NKT * j + kt:NKT * j + kt + 1],
                    p_raw[:], AL.is_ge, AL.mult)
                for dvc in range(2):
                    nc.tensor.matmul(
                        o_ps[:, dvc, :],
                        vt_sb[:, kt, 128 * dvc:128 * (dvc + 1)],
                        p_m[:],
                        start=(kt == 0), stop=(kt == E - 1))
                nc.tensor.matmul(
                    l_ps[:], ones_sb[:], p_m[:],
                    start=(kt == 0), stop=(kt == E - 1))
            o_sb = p_pool.tile([128, 2, QW], f32, tag="osb")
            l_sb = p_pool.tile([1, QW], f32, tag="lsb")
            nc.vector.tensor_copy(o_sb[:], o_ps[:])
            nc.vector.tensor_copy(l_sb[:], l_ps[:])
            for dvc in range(2):
                nc.sync.dma_start(
                    o_t[j, 128 * dvc:128 * (dvc + 1), :], o_sb[:, dvc, :])
            nc.sync.dma_start(l_o[j], l_sb[:])

    nc.compile()
    return nc


def _prep_inputs(inputs, Wq, bq, Wk, bk, Wv, bv):
    import ml_dtypes
    bf16 = ml_dtypes.bfloat16
    in_maps = []
    xt_b = [np.ascontiguousarray(inputs[b].T).astype(bf16) for b in range(B)]
    wq_ = Wq.astype(bf16)
    wk_ = Wk.astype(bf16)
    wv_ = Wv.astype(bf16)
    bqk = np.stack([bq[:128], bq[128:], bk[:128], bk[128:]],
                   axis=1).astype(np.float32)
    bvb = np.tile(bv[None, :], (128, 1)).astype(np.float32)
    colio = np.tile(np.arange(QW, dtype=np.float32)[None, :], (128, 1))
    ones = np.ones((128, 1), dtype=bf16)
    for core in range(8):
        b, h = core % B, core // B
        chunks = CHUNKS_H[h]
        q0s = [QW * c for c in chunks]
        xq = np.concatenate([xt_b[b][:, q0:q0 + QW] for q0 in q0s], axis=1)
        thr_np = np.empty((128, NSLOT * NKT), dtype=np.float32)
        r = np.arange(128, dtype=np.float32)
        for j in range(NSLOT):
            for kt in range(NKT):
                thr_np[:, NKT * j + kt] = 128 * kt + r - q0s[j]
        in_maps.append({
            "xt": xt_b[b], "xq": np.ascontiguousarray(xq),
            "wq": wq_, "wk": wk_, "wv": wv_,
            "bqk": bqk, "bvb": bvb, "colio": colio,
            "thr": thr_np, "ones": ones,
        })
    return in_maps


def kernel(inputs, Wq, bq, Wk, bk, Wv, bv):
    from concourse.bass_utils import run_bass_kernel_spmd

    if "nc" not in _CACHE:
        _CACHE["nc"] = _build_graph()
    nc = _CACHE["nc"]

    in_maps = _prep_inputs(
        np.asarray(inputs), np.asarray(Wq), np.asarray(bq), np.asarray(Wk),
        np.asarray(bk), np.asarray(Wv), np.asarray(bv))

    res = run_bass_kernel_spmd(nc, in_maps, core_ids=list(range(8)))
    _CACHE["last_results"] = res

    out = np.empty((B, T, DK), dtype=np.float32)
    for core in range(8):
        b, h = core % B, core // B
        r = res.results[core]
        o_t, l_v = np.asarray(r["o_t"]), np.asarray(r["l_o"])
        for j, c in enumerate(CHUNKS_H[h]):
            q0 = QW * c
            out[b, q0:q0 + QW, :] = (o_t[j] / l_v[j]).T
    return out


if __name__ == "__main__":
    import reference
    ins = {k: np.asarray(v) for k, v in reference.setup_inputs().items()}
    exp = np.asarray(reference.reference(**{k: v for k, v in ins.items()}))
    act = kernel(**ins)
    err = np.linalg.norm(act - exp) / np.linalg.norm(exp)
    print("Relative error:", err)


# revision 5
# speedup vs baseline: 1.1865x; 1.1865x over previous
"""Causal attention (B=4,T=4096,Dm=1024,Dk=256) on 8 TRN2 NeuronCores.

Sharding: 8 cores = 4 batches x 2 query-groups. Query dim split into eight
512-wide chunks; group h=0 takes chunks (7,5,2,0), h=1 takes (6,4,3,1) so
that causal work balances. Each core runs an IDENTICAL graph with 4 query
slots of uniform key-tile extents (32,24,16,8); the causal mask (incl. the
padding beyond a chunk's real causal extent) is pure data: a per-(slot,
key-tile) threshold column and `(col_iota >= thr) * exp(S/16)` fused on DVE.

Everything on-chip is computed in the transposed layout (Q^T,K^T,S^T,O^T)
so no PE transposes are needed; softmax is unnormalized (no max subtraction
-- scores/16 are O(1)) and the division by l plus the final transpose is
done on host. Compute dtype bf16, accumulation fp32.
"""

import math
import numpy as np
from contextlib import ExitStack

B, T, DM, DK = 4, 4096, 1024, 256
QW = 512                      # query chunk width
NSLOT = 4                     # query slots per core
SLOT_EXT = (32, 24, 16, 8)    # key-tile extent per slot (uniform across cores)
CHUNKS_H = {0: (7, 5, 2, 0), 1: (6, 4, 3, 1)}  # chunk idx per slot, per group
NKT = T // 128                # 32 key tiles
SCALE = 1.0 / math.sqrt(DK)   # 1/16

_CACHE = {}


def _build_graph():
    from concourse import bacc, mybir, tile

    f32 = mybir.dt.float32
    bf16 = mybir.dt.bfloat16
    AL = mybir.AluOpType

    nc = bacc.Bacc(None, target_bir_lowering=False)
    xt = nc.declare_dram_parameter("xt", [DM, T], bf16, isOutput=False)
    xq = nc.declare_dram_parameter("xq", [DM, NSLOT * QW], bf16, isOutput=False)
    wq = nc.declare_dram_parameter("wq", [DM, DK], bf16, isOutput=False)
    wk = nc.declare_dram_parameter("wk", [DM, DK], bf16, isOutput=False)
    wv = nc.declare_dram_parameter("wv", [DM, DK], bf16, isOutput=False)
    bqk = nc.declare_dram_parameter("bqk", [128, 4], f32, isOutput=False)
    bvb = nc.declare_dram_parameter("bvb", [128, DK], f32, isOutput=False)
    colio = nc.declare_dram_parameter("colio", [128, QW], f32, isOutput=False)
    thr = nc.declare_dram_parameter("thr", [128, NSLOT * NKT], f32, isOutput=False)
    ones = nc.declare_dram_parameter("ones", [128, 1], bf16, isOutput=False)
    o_t = nc.declare_dram_parameter("o_t", [NSLOT, DK, QW], f32, isOutput=True)
    l_o = nc.declare_dram_parameter("l_o", [NSLOT, 1, QW], f32, isOutput=True)

    CB = T // 4  # 1024-column DMA/projection blocks

    with tile.TileContext(nc) as tc, ExitStack() as ctx:
        const = ctx.enter_context(tc.tile_pool(name="const", bufs=1))
        xt_pool = ctx.enter_context(tc.tile_pool(name="xt_pool", bufs=32))
        xq_pool = ctx.enter_context(tc.tile_pool(name="xq_pool", bufs=8))
        kvq = ctx.enter_context(tc.tile_pool(name="kvq", bufs=1))
        p_pool = ctx.enter_context(tc.tile_pool(name="p_pool", bufs=4))
        ps_pool = ctx.enter_context(
            tc.tile_pool(name="ps_pool", bufs=4, space="PSUM"))
        o_pool = ctx.enter_context(
            tc.tile_pool(name="o_pool", bufs=1, space="PSUM"))
        l_pool = ctx.enter_context(
            tc.tile_pool(name="l_pool", bufs=2, space="PSUM"))

        # constants / weights
        wq_sb = const.tile([128, 8, DK], bf16, tag="wq")
        wk_sb = const.tile([128, 8, DK], bf16, tag="wk")
        wv_sb = const.tile([128, 8, DK], bf16, tag="wv")
        bqk_sb = const.tile([128, 4], f32, tag="bqk")
        bvb_sb = const.tile([128, DK], f32, tag="bvb")
        colio_sb = const.tile([128, QW], f32, tag="colio")
        thr_sb = const.tile([128, NSLOT * NKT], f32, tag="thr")
        ones_sb = const.tile([128, 1], bf16, tag="ones")
        nc.sync.dma_start(bqk_sb[:], bqk[:])
        nc.sync.dma_start(bvb_sb[:], bvb[:])
        nc.sync.dma_start(colio_sb[:], colio[:])
        nc.sync.dma_start(thr_sb[:], thr[:])
        nc.sync.dma_start(ones_sb[:], ones[:])
        nc.sync.dma_start(wq_sb[:], wq.rearrange("(c p) n -> p c n", p=128))
        nc.sync.dma_start(wk_sb[:], wk.rearrange("(c p) n -> p c n", p=128))
        nc.sync.dma_start(wv_sb[:], wv.rearrange("(c p) n -> p c n", p=128))

        # activations: xq first (small, needed by Q^T), xt column-blocked
        xq_c = []
        for c in range(8):
            t_ = xq_pool.tile([128, NSLOT * QW], bf16, tag="xq")
            nc.sync.dma_start(t_[:], xq[128 * c:128 * (c + 1), :])
            xq_c.append(t_)
        xt_cb = {}  # (c, cb) -> [128, CB] tile
        for cb in range(4):
            for c in range(8):
                t_ = xt_pool.tile([128, CB], bf16, tag="xt")
                nc.sync.dma_start(
                    t_[:], xt[128 * c:128 * (c + 1), CB * cb:CB * (cb + 1)])
                xt_cb[(c, cb)] = t_

        kt_sb = kvq.tile([128, 2, T], bf16, tag="kt")       # K^T
        vt_sb = kvq.tile([128, NKT, DK], bf16, tag="vt")    # V  (Ts-part)
        qt_sb = kvq.tile([128, 2, NSLOT * QW], bf16, tag="qt")  # Q^T

        # ---- Q^T projection (first: xq is small and arrives early) ----
        for dkc in range(2):
            for tj in range(NSLOT):
                ps = ps_pool.tile([128, QW], f32, tag="ps")
                for c in range(8):
                    nc.tensor.matmul(
                        ps[:],
                        wq_sb[:, c, 128 * dkc:128 * (dkc + 1)],
                        xq_c[c][:, QW * tj:QW * (tj + 1)],
                        start=(c == 0), stop=(c == 7))
                nc.vector.tensor_scalar(
                    qt_sb[:, dkc, QW * tj:QW * (tj + 1)], ps[:],
                    bqk_sb[:, dkc:dkc + 1], None, AL.add)

        def proj_kv_block(cb):
            # K^T columns [CB*cb, CB*(cb+1)) and V tiles 8cb..8cb+7
            for dkc in range(2):
                for half in range(2):
                    tj = 2 * cb + half
                    ps = ps_pool.tile([128, QW], f32, tag="ps")
                    for c in range(8):
                        nc.tensor.matmul(
                            ps[:],
                            wk_sb[:, c, 128 * dkc:128 * (dkc + 1)],
                            xt_cb[(c, cb)][:, QW * half:QW * (half + 1)],
                            start=(c == 0), stop=(c == 7))
                    nc.vector.tensor_scalar(
                        kt_sb[:, dkc, QW * tj:QW * (tj + 1)], ps[:],
                        bqk_sb[:, 2 + dkc:3 + dkc], None, AL.add)
            for tl in range(8):
                tt = 8 * cb + tl
                ps = ps_pool.tile([128, QW], f32, tag="ps")
                for c in range(8):
                    nc.tensor.matmul(
                        ps[:, :DK],
                        xt_cb[(c, cb)][:, 128 * tl:128 * (tl + 1)],
                        wv_sb[:, c, :],
                        start=(c == 0), stop=(c == 7))
                nc.vector.tensor_tensor(
                    vt_sb[:, tt, :], ps[:, :DK], bvb_sb[:], AL.add)

        def attn_slot(j):
            E = SLOT_EXT[j]
            o_ps = o_pool.tile([128, 2, QW], f32, tag="o")
            l_ps = l_pool.tile([1, QW], f32, tag="l")
            for kt in range(E):
                s_ps = ps_pool.tile([128, QW], f32, tag="ps")
                for dkc in range(2):
                    nc.tensor.matmul(
                        s_ps[:],
                        kt_sb[:, dkc, 128 * kt:128 * (kt + 1)],
                        qt_sb[:, dkc, QW * j:QW * (j + 1)],
                        start=(dkc == 0), stop=(dkc == 1))
                p_raw = p_pool.tile([128, QW], bf16, tag="praw")
                nc.scalar.activation(
                    p_raw[:], s_ps[:],
                    mybir.ActivationFunctionType.Exp, scale=SCALE)
                if kt >= E - 8:
                    # only the last 8 tiles of a slot can be causal-cut or
                    # padding (verified for both core groups); others are
                    # provably full -> skip the mask multiply
                    p_m = p_pool.tile([128, QW], bf16, tag="pm")
                    nc.vector.scalar_tensor_tensor(
                        p_m[:], colio_sb[:],
                        thr_sb[:, NKT * j + kt:NKT * j + kt + 1],
                        p_raw[:], AL.is_ge, AL.mult)
                else:
                    p_m = p_raw
                for dvc in range(2):
                    nc.tensor.matmul(
                        o_ps[:, dvc, :],
                        vt_sb[:, kt, 128 * dvc:128 * (dvc + 1)],
                        p_m[:],
                        start=(kt == 0), stop=(kt == E - 1))
                nc.tensor.matmul(
                    l_ps[:], ones_sb[:], p_m[:],
                    start=(kt == 0), stop=(kt == E - 1))
            o_sb = p_pool.tile([128, 2, QW], f32, tag="osb")
            l_sb = p_pool.tile([1, QW], f32, tag="lsb")
            nc.vector.tensor_copy(o_sb[:], o_ps[:])
            nc.vector.tensor_copy(l_sb[:], l_ps[:])
            for dvc in range(2):
                nc.sync.dma_start(
                    o_t[j, 128 * dvc:128 * (dvc + 1), :], o_sb[:, dvc, :])
            nc.sync.dma_start(l_o[j], l_sb[:])

        # interleave: after col-block cb is projected, slot 3-cb (extent
        # 8*(cb+1)) has exactly the K/V tiles it needs
        for cb in range(4):
            proj_kv_block(cb)
            attn_slot(3 - cb)

    nc.compile()
    return nc


def _prep_inputs(inputs, Wq, bq, Wk, bk, Wv, bv):
    import ml_dtypes
    bf16 = ml_dtypes.bfloat16
    in_maps = []
    xt_b = [np.ascontiguousarray(inputs[b].T).astype(bf16) for b in range(B)]
    wq_ = Wq.astype(bf16)
    wk_ = Wk.astype(bf16)
    wv_ = Wv.astype(bf16)
    bqk = np.stack([bq[:128], bq[128:], bk[:128], bk[128:]],
                   axis=1).astype(np.float32)
    bvb = np.tile(bv[None, :], (128, 1)).astype(np.float32)
    colio = np.tile(np.arange(QW, dtype=np.float32)[None, :], (128, 1))
    ones = np.ones((128, 1), dtype=bf16)
    for core in range(8):
        b, h = core % B, core // B
        chunks = CHUNKS_H[h]
        q0s = [QW * c for c in chunks]
        xq = np.concatenate([xt_b[b][:, q0:q0 + QW] for q0 in q0s], axis=1)
        thr_np = np.empty((128, NSLOT * NKT), dtype=np.float32)
        r = np.arange(128, dtype=np.float32)
        for j in range(NSLOT):
            for kt in range(NKT):
                thr_np[:, NKT * j + kt] = 128 * kt + r - q0s[j]
        in_maps.append({
            "xt": xt_b[b], "xq": np.ascontiguousarray(xq),
            "wq": wq_, "wk": wk_, "wv": wv_,
            "bqk": bqk, "bvb": bvb, "colio": colio,
            "thr": thr_np, "ones": ones,
        })
    return in_maps


def kernel(inputs, Wq, bq, Wk, bk, Wv, bv):
    from concourse.bass_utils import run_bass_kernel_spmd

    if "nc" not in _CACHE:
        _CACHE["nc"] = _build_graph()
    nc = _CACHE["nc"]

    in_maps = _prep_inputs(
        np.asarray(inputs), np.asarray(Wq), np.asarray(bq), np.asarray(Wk),
        np.asarray(bk), np.asarray(Wv), np.asarray(bv))

    res = run_bass_kernel_spmd(nc, in_maps, core_ids=list(range(8)))
    _CACHE["last_results"] = res

    out = np.empty((B, T, DK), dtype=np.float32)
    for core in range(8):
        b, h = core % B, core // B
        r = res.results[core]
        o_t, l_v = np.asarray(r["o_t"]), np.asarray(r["l_o"])
        for j, c in enumerate(CHUNKS_H[h]):
            q0 = QW * c
            out[b, q0:q0 + QW, :] = (o_t[j] / l_v[j]).T
    return out


if __name__ == "__main__":
    import reference
    ins = {k: np.asarray(v) for k, v in reference.setup_inputs().items()}
    exp = np.asarray(reference.reference(**{k: v for k, v in ins.items()}))
    act = kernel(**ins)
    err = np.linalg.norm(act - exp) / np.linalg.norm(exp)
    print("Relative error:", err)


# revision 12
# speedup vs baseline: 1.3488x; 1.1368x over previous
"""Causal attention (B=4,T=4096,Dm=1024,Dk=256) on 8 TRN2 NeuronCores.

Sharding: 8 cores = 4 batches x 2 query-groups. Query dim split into eight
512-wide chunks; group h=0 takes chunks (7,5,2,0), h=1 takes (6,4,3,1) so
that causal work balances. Each core runs an IDENTICAL graph with 4 query
slots of uniform key-tile extents (32,24,16,8); the causal mask (incl. the
padding beyond a chunk's real causal extent) is pure data: a per-(slot,
key-tile) threshold column and `(col_iota >= thr) * exp(S/16)` fused on DVE.

Everything on-chip is computed in the transposed layout (Q^T,K^T,S^T,O^T)
so no PE transposes are needed; softmax is unnormalized (no max subtraction
-- scores/16 are O(1)) and the division by l plus the final transpose is
done on host. Compute dtype bf16, accumulation fp32.
"""

import math
import numpy as np
from contextlib import ExitStack

B, T, DM, DK = 4, 4096, 1024, 256
QW = 512                      # query chunk width
NSLOT = 4                     # query slots per core
SLOT_EXT = (32, 24, 16, 8)    # key-tile extent per slot (uniform across cores)
CHUNKS_H = {0: (7, 5, 2, 0), 1: (6, 4, 3, 1)}  # chunk idx per slot, per group
NKT = T // 128                # 32 key tiles
SCALE = 1.0 / math.sqrt(DK)   # 1/16

_CACHE = {}


def _build_graph():
    from concourse import bacc, mybir, tile

    f32 = mybir.dt.float32
    bf16 = mybir.dt.bfloat16
    AL = mybir.AluOpType

    nc = bacc.Bacc(None, target_bir_lowering=False)
    xt = nc.declare_dram_parameter("xt", [DM, T], bf16, isOutput=False)
    xq = nc.declare_dram_parameter("xq", [DM, NSLOT * QW], bf16, isOutput=False)
    wq = nc.declare_dram_parameter("wq", [DM, DK], bf16, isOutput=False)
    wk = nc.declare_dram_parameter("wk", [DM, DK], bf16, isOutput=False)
    wv = nc.declare_dram_parameter("wv", [DM, DK], bf16, isOutput=False)
    # consts packed into one f32 tensor: [bqk(4) | bvb(256) | colio(512) |
    # thr(128)] along the free dim
    NCONST = 4 + DK + QW + NSLOT * NKT
    cst = nc.declare_dram_parameter("cst", [128, NCONST], f32, isOutput=False)
    o_t = nc.declare_dram_parameter("o_t", [NSLOT, DK, QW], f32, isOutput=True)
    l_o = nc.declare_dram_parameter("l_o", [NSLOT, 1, QW], f32, isOutput=True)

    CB = T // 4  # 1024-column DMA/projection blocks

    with tile.TileContext(nc) as tc, ExitStack() as ctx:
        const = ctx.enter_context(tc.tile_pool(name="const", bufs=1))
        xt_pool = ctx.enter_context(tc.tile_pool(name="xt_pool", bufs=32))
        xq_pool = ctx.enter_context(tc.tile_pool(name="xq_pool", bufs=8))
        kvq = ctx.enter_context(tc.tile_pool(name="kvq", bufs=1))
        p_pool = ctx.enter_context(tc.tile_pool(name="p_pool", bufs=4))
        ps_pool = ctx.enter_context(
            tc.tile_pool(name="ps_pool", bufs=5, space="PSUM"))
        o_pool = ctx.enter_context(
            tc.tile_pool(name="o_pool", bufs=1, space="PSUM"))
        l_pool = ctx.enter_context(
            tc.tile_pool(name="l_pool", bufs=1, space="PSUM"))

        # constants / weights on the scalar HWDGE queue; activations on the
        # sync queue -- two parallel DMA issue streams shrink the head
        cst_sb = const.tile([128, NCONST], f32, tag="cst")
        nc.scalar.dma_start(cst_sb[:], cst[:])
        bqk_sb = cst_sb[:, 0:4]
        bvb_sb = cst_sb[:, 4:4 + DK]
        colio_sb = cst_sb[:, 4 + DK:4 + DK + QW]
        thr_sb = cst_sb[:, 4 + DK + QW:]
        wq_sb = const.tile([128, 8, DK], bf16, tag="wq")
        wk_sb = const.tile([128, 8, DK], bf16, tag="wk")
        wv_sb = const.tile([128, 8, DK], bf16, tag="wv")
        nc.scalar.dma_start(wq_sb[:], wq.rearrange("(c p) n -> p c n", p=128))
        nc.scalar.dma_start(wk_sb[:], wk.rearrange("(c p) n -> p c n", p=128))
        nc.scalar.dma_start(wv_sb[:], wv.rearrange("(c p) n -> p c n", p=128))
        ones_sb = const.tile([128, 1], bf16, tag="ones")
        nc.vector.memset(ones_sb[:], 1.0)

        # activations: xq first (small, needed by Q^T), xt column-blocked
        xq_c = []
        for c in range(8):
            t_ = xq_pool.tile([128, NSLOT * QW], bf16, tag="xq")
            eng = nc.sync if c < 4 else nc.scalar
            eng.dma_start(t_[:], xq[128 * c:128 * (c + 1), :])
            xq_c.append(t_)
        xt_cb = {}  # (c, cb) -> [128, CB] tile
        for cb in range(4):
            for c in range(8):
                t_ = xt_pool.tile([128, CB], bf16, tag="xt")
                nc.sync.dma_start(
                    t_[:], xt[128 * c:128 * (c + 1), CB * cb:CB * (cb + 1)])
                xt_cb[(c, cb)] = t_

        kt_sb = kvq.tile([128, 2, T], bf16, tag="kt")       # K^T
        vt_sb = kvq.tile([128, NKT, DK], bf16, tag="vt")    # V  (Ts-part)
        qt_sb = kvq.tile([128, 2, NSLOT * QW], bf16, tag="qt")  # Q^T

        # ---- Q^T projection (first: xq is small and arrives early) ----
        for dkc in range(2):
            for tj in range(NSLOT):
                ps = ps_pool.tile([128, QW], f32, tag="ps")
                for c in range(8):
                    nc.tensor.matmul(
                        ps[:],
                        wq_sb[:, c, 128 * dkc:128 * (dkc + 1)],
                        xq_c[c][:, QW * tj:QW * (tj + 1)],
                        start=(c == 0), stop=(c == 7))
                nc.vector.tensor_scalar(
                    qt_sb[:, dkc, QW * tj:QW * (tj + 1)], ps[:],
                    bqk_sb[:, dkc:dkc + 1], None, AL.add)

        def proj_kv_block(cb):
            # K^T columns [CB*cb, CB*(cb+1)) and V tiles 8cb..8cb+7
            for dkc in range(2):
                for half in range(2):
                    tj = 2 * cb + half
                    ps = ps_pool.tile([128, QW], f32, tag="ps")
                    for c in range(8):
                        nc.tensor.matmul(
                            ps[:],
                            wk_sb[:, c, 128 * dkc:128 * (dkc + 1)],
                            xt_cb[(c, cb)][:, QW * half:QW * (half + 1)],
                            start=(c == 0), stop=(c == 7))
                    nc.vector.tensor_scalar(
                        kt_sb[:, dkc, QW * tj:QW * (tj + 1)], ps[:],
                        bqk_sb[:, 2 + dkc:3 + dkc], None, AL.add)
            for tl in range(8):
                tt = 8 * cb + tl
                ps = ps_pool.tile([128, QW], f32, tag="ps")
                for c in range(8):
                    nc.tensor.matmul(
                        ps[:, :DK],
                        xt_cb[(c, cb)][:, 128 * tl:128 * (tl + 1)],
                        wv_sb[:, c, :],
                        start=(c == 0), stop=(c == 7))
                nc.vector.tensor_tensor(
                    vt_sb[:, tt, :], ps[:, :DK], bvb_sb[:], AL.add)

        def attn_slot(j):
            E = SLOT_EXT[j]
            o_ps = o_pool.tile([128, 2, QW], f32, tag="o")
            l_ps = l_pool.tile([1, QW], f32, tag="l")
            p_acc = None
            for kt in range(E):
                s_ps = ps_pool.tile([128, QW], f32, tag="ps")
                for dkc in range(2):
                    nc.tensor.matmul(
                        s_ps[:],
                        kt_sb[:, dkc, 128 * kt:128 * (kt + 1)],
                        qt_sb[:, dkc, QW * j:QW * (j + 1)],
                        start=(dkc == 0), stop=(dkc == 1))
                p_raw = p_pool.tile([128, QW], bf16, tag="praw")
                nc.scalar.activation(
                    p_raw[:], s_ps[:],
                    mybir.ActivationFunctionType.Exp, scale=SCALE)
                if kt >= E - 8:
                    # only the last 8 tiles of a slot can be causal-cut or
                    # padding (verified for both core groups); others are
                    # provably full -> skip the mask multiply
                    p_m = p_pool.tile([128, QW], bf16, tag="pm")
                    nc.vector.scalar_tensor_tensor(
                        p_m[:], colio_sb[:],
                        thr_sb[:, NKT * j + kt:NKT * j + kt + 1],
                        p_raw[:], AL.is_ge, AL.mult)
                else:
                    p_m = p_raw
                for dvc in range(2):
                    nc.tensor.matmul(
                        o_ps[:, dvc, :],
                        vt_sb[:, kt, 128 * dvc:128 * (dvc + 1)],
                        p_m[:],
                        start=(kt == 0), stop=(kt == E - 1))
                # l: sum 4 P tiles on DVE, one ones-matmul per quad (4x
                # less PE than per-tile lsum; bf16 accum err ~0.2%, fine)
                if kt % 4 == 0:
                    p_acc = p_pool.tile([128, QW], bf16, tag="pacc")
                    nc.vector.tensor_copy(p_acc[:], p_m[:])
                else:
                    nc.vector.tensor_tensor(p_acc[:], p_acc[:], p_m[:], AL.add)
                if kt % 4 == 3:
                    nc.tensor.matmul(
                        l_ps[:], ones_sb[:], p_acc[:],
                        start=(kt == 3), stop=(kt == E - 1))
            o_sb = p_pool.tile([128, 2, QW], f32, tag="osb")
            l_sb = p_pool.tile([1, QW], f32, tag="lsb")
            nc.vector.tensor_copy(o_sb[:], o_ps[:])
            nc.vector.tensor_copy(l_sb[:], l_ps[:])
            for dvc in range(2):
                nc.sync.dma_start(
                    o_t[j, 128 * dvc:128 * (dvc + 1), :], o_sb[:, dvc, :])
            nc.sync.dma_start(l_o[j], l_sb[:])

        # interleave: after col-block cb is projected, slot 3-cb (extent
        # 8*(cb+1)) has exactly the K/V tiles it needs
        for cb in range(4):
            proj_kv_block(cb)
            attn_slot(3 - cb)

    nc.compile()
    return nc


def _prep_inputs(inputs, Wq, bq, Wk, bk, Wv, bv):
    import ml_dtypes
    bf16 = ml_dtypes.bfloat16
    in_maps = []
    xt_b = [np.ascontiguousarray(inputs[b].T).astype(bf16) for b in range(B)]
    wq_ = Wq.astype(bf16)
    wk_ = Wk.astype(bf16)
    wv_ = Wv.astype(bf16)
    bqk = np.stack([bq[:128], bq[128:], bk[:128], bk[128:]],
                   axis=1).astype(np.float32)
    bvb = np.tile(bv[None, :], (128, 1)).astype(np.float32)
    colio = np.tile(np.arange(QW, dtype=np.float32)[None, :], (128, 1))
    for core in range(8):
        b, h = core % B, core // B
        chunks = CHUNKS_H[h]
        q0s = [QW * c for c in chunks]
        xq = np.concatenate([xt_b[b][:, q0:q0 + QW] for q0 in q0s], axis=1)
        thr_np = np.empty((128, NSLOT * NKT), dtype=np.float32)
        r = np.arange(128, dtype=np.float32)
        for j in range(NSLOT):
            for kt in range(NKT):
                thr_np[:, NKT * j + kt] = 128 * kt + r - q0s[j]
        cst = np.concatenate([bqk, bvb, colio, thr_np], axis=1)
        in_maps.append({
            "xt": xt_b[b], "xq": np.ascontiguousarray(xq),
            "wq": wq_, "wk": wk_, "wv": wv_,
            "cst": np.ascontiguousarray(cst),
        })
    return in_maps


def kernel(inputs, Wq, bq, Wk, bk, Wv, bv):
    from concourse.bass_utils import run_bass_kernel_spmd

    if "nc" not in _CACHE:
        _CACHE["nc"] = _build_graph()
    nc = _CACHE["nc"]

    in_maps = _prep_inputs(
        np.asarray(inputs), np.asarray(Wq), np.asarray(bq), np.asarray(Wk),
        np.asarray(bk), np.asarray(Wv), np.asarray(bv))

    res = run_bass_kernel_spmd(nc, in_maps, core_ids=list(range(8)))
    _CACHE["last_results"] = res

    out = np.empty((B, T, DK), dtype=np.float32)
    for core in range(8):
        b, h = core % B, core // B
        r = res.results[core]
        o_t, l_v = np.asarray(r["o_t"]), np.asarray(r["l_o"])
        for j, c in enumerate(CHUNKS_H[h]):
            q0 = QW * c
            out[b, q0:q0 + QW, :] = (o_t[j] / l_v[j]).T
    return out


if __name__ == "__main__":
    import reference
    ins = {k: np.asarray(v) for k, v in reference.setup_inputs().items()}
    exp = np.asarray(reference.reference(**{k: v for k, v in ins.items()}))
    act = kernel(**ins)
    err = np.linalg.norm(act - exp) / np.linalg.norm(exp)
    print("Relative error:", err)


# revision 15
# speedup vs baseline: 1.3602x; 1.0084x over previous
"""Causal attention (B=4,T=4096,Dm=1024,Dk=256) on 8 TRN2 NeuronCores.

Sharding: 8 cores = 4 batches x 2 query-groups. Query dim split into eight
512-wide chunks; group h=0 takes chunks (7,5,2,0), h=1 takes (6,4,3,1) so
that causal work balances. Each core runs an IDENTICAL graph with 4 query
slots of uniform key-tile extents (32,24,16,8); the causal mask (incl. the
padding beyond a chunk's real causal extent) is pure data: a per-(slot,
key-tile) threshold column and `(col_iota >= thr) * exp(S/16)` fused on DVE.

Everything on-chip is computed in the transposed layout (Q^T,K^T,S^T,O^T)
so no PE transposes are needed; softmax is unnormalized (no max subtraction
-- scores/16 are O(1)) and the division by l plus the final transpose is
done on host. Compute dtype bf16, accumulation fp32.
"""

import math
import numpy as np
from contextlib import ExitStack

B, T, DM, DK = 4, 4096, 1024, 256
QW = 512                      # query chunk width
NSLOT = 4                     # query slots per core
SLOT_EXT = (32, 24, 16, 8)    # key-tile extent per slot (uniform across cores)
CHUNKS_H = {0: (7, 5, 2, 0), 1: (6, 4, 3, 1)}  # chunk idx per slot, per group
NKT = T // 128                # 32 key tiles
SCALE = 1.0 / math.sqrt(DK)   # 1/16

_CACHE = {}


def _build_graph():
    from concourse import bacc, mybir, tile

    f32 = mybir.dt.float32
    bf16 = mybir.dt.bfloat16
    AL = mybir.AluOpType

    nc = bacc.Bacc(None, target_bir_lowering=False)
    xt = nc.declare_dram_parameter("xt", [DM, T], bf16, isOutput=False)
    xq = nc.declare_dram_parameter("xq", [DM, NSLOT * QW], bf16, isOutput=False)
    wq = nc.declare_dram_parameter("wq", [DM, DK], bf16, isOutput=False)
    wk = nc.declare_dram_parameter("wk", [DM, DK], bf16, isOutput=False)
    wv = nc.declare_dram_parameter("wv", [DM, DK], bf16, isOutput=False)
    # consts packed into one f32 tensor: [bqk(4) | bvb(256) | colio(512) |
    # thr(128)] along the free dim
    NCONST = 4 + DK + QW + NSLOT * NKT
    cst = nc.declare_dram_parameter("cst", [128, NCONST], f32, isOutput=False)
    o_t = nc.declare_dram_parameter("o_t", [NSLOT, DK, QW], f32, isOutput=True)
    l_o = nc.declare_dram_parameter("l_o", [NSLOT, 1, QW], f32, isOutput=True)

    CB = T // 4  # 1024-column DMA/projection blocks

    with tile.TileContext(nc) as tc, ExitStack() as ctx:
        const = ctx.enter_context(tc.tile_pool(name="const", bufs=1))
        xt_pool = ctx.enter_context(tc.tile_pool(name="xt_pool", bufs=32))
        xq_pool = ctx.enter_context(tc.tile_pool(name="xq_pool", bufs=8))
        kvq = ctx.enter_context(tc.tile_pool(name="kvq", bufs=1))
        p_pool = ctx.enter_context(tc.tile_pool(name="p_pool", bufs=4))
        ps_pool = ctx.enter_context(
            tc.tile_pool(name="ps_pool", bufs=5, space="PSUM"))
        o_pool = ctx.enter_context(
            tc.tile_pool(name="o_pool", bufs=1, space="PSUM"))
        l_pool = ctx.enter_context(
            tc.tile_pool(name="l_pool", bufs=1, space="PSUM"))

        # constants / weights on the scalar HWDGE queue; activations on the
        # sync queue -- two parallel DMA issue streams shrink the head
        cst_sb = const.tile([128, NCONST], f32, tag="cst")
        nc.scalar.dma_start(cst_sb[:], cst[:])
        bqk_sb = cst_sb[:, 0:4]
        bvb_sb = cst_sb[:, 4:4 + DK]
        colio_sb = cst_sb[:, 4 + DK:4 + DK + QW]
        thr_sb = cst_sb[:, 4 + DK + QW:]
        wq_sb = const.tile([128, 8, DK], bf16, tag="wq")
        wk_sb = const.tile([128, 8, DK], bf16, tag="wk")
        wv_sb = const.tile([128, 8, DK], bf16, tag="wv")
        nc.scalar.dma_start(wk_sb[:], wk.rearrange("(c p) n -> p c n", p=128))
        nc.scalar.dma_start(wv_sb[:], wv.rearrange("(c p) n -> p c n", p=128))
        nc.scalar.dma_start(wq_sb[:], wq.rearrange("(c p) n -> p c n", p=128))
        ones_sb = const.tile([128, 1], bf16, tag="ones")
        nc.vector.memset(ones_sb[:], 1.0)

        # DMA priority: xt col-block 0 first (K/V proj of cb0 is the first
        # PE work), then xq (Q^T), then the remaining xt col-blocks
        xt_cb = {}  # (c, cb) -> [128, CB] tile
        for c in range(8):
            t_ = xt_pool.tile([128, CB], bf16, tag="xt")
            nc.sync.dma_start(t_[:], xt[128 * c:128 * (c + 1), 0:CB])
            xt_cb[(c, 0)] = t_
        xq_c = []
        for c in range(8):
            t_ = xq_pool.tile([128, NSLOT * QW], bf16, tag="xq")
            eng = nc.sync if c < 4 else nc.scalar
            eng.dma_start(t_[:], xq[128 * c:128 * (c + 1), :])
            xq_c.append(t_)
        for cb in range(1, 4):
            for c in range(8):
                t_ = xt_pool.tile([128, CB], bf16, tag="xt")
                nc.sync.dma_start(
                    t_[:], xt[128 * c:128 * (c + 1), CB * cb:CB * (cb + 1)])
                xt_cb[(c, cb)] = t_

        kt_sb = kvq.tile([128, 2, T], bf16, tag="kt")       # K^T
        vt_sb = kvq.tile([128, NKT, DK], bf16, tag="vt")    # V  (Ts-part)
        qt_sb = kvq.tile([128, 2, NSLOT * QW], bf16, tag="qt")  # Q^T

        def proj_q():
            for dkc in range(2):
                for tj in range(NSLOT):
                    ps = ps_pool.tile([128, QW], f32, tag="ps")
                    for c in range(8):
                        nc.tensor.matmul(
                            ps[:],
                            wq_sb[:, c, 128 * dkc:128 * (dkc + 1)],
                            xq_c[c][:, QW * tj:QW * (tj + 1)],
                            start=(c == 0), stop=(c == 7))
                    nc.vector.tensor_scalar(
                        qt_sb[:, dkc, QW * tj:QW * (tj + 1)], ps[:],
                        bqk_sb[:, dkc:dkc + 1], None, AL.add)

        def proj_kv_block(cb):
            # K^T columns [CB*cb, CB*(cb+1)) and V tiles 8cb..8cb+7
            for dkc in range(2):
                for half in range(2):
                    tj = 2 * cb + half
                    ps = ps_pool.tile([128, QW], f32, tag="ps")
                    for c in range(8):
                        nc.tensor.matmul(
                            ps[:],
                            wk_sb[:, c, 128 * dkc:128 * (dkc + 1)],
                            xt_cb[(c, cb)][:, QW * half:QW * (half + 1)],
                            start=(c == 0), stop=(c == 7))
                    nc.vector.tensor_scalar(
                        kt_sb[:, dkc, QW * tj:QW * (tj + 1)], ps[:],
                        bqk_sb[:, 2 + dkc:3 + dkc], None, AL.add)
            for tl in range(8):
                tt = 8 * cb + tl
                ps = ps_pool.tile([128, QW], f32, tag="ps")
                for c in range(8):
                    nc.tensor.matmul(
                        ps[:, :DK],
                        xt_cb[(c, cb)][:, 128 * tl:128 * (tl + 1)],
                        wv_sb[:, c, :],
                        start=(c == 0), stop=(c == 7))
                nc.vector.tensor_tensor(
                    vt_sb[:, tt, :], ps[:, :DK], bvb_sb[:], AL.add)

        def attn_slot(j):
            E = SLOT_EXT[j]
            o_ps = o_pool.tile([128, 2, QW], f32, tag="o")
            l_ps = l_pool.tile([1, QW], f32, tag="l")
            p_acc = None
            for kt in range(E):
                s_ps = ps_pool.tile([128, QW], f32, tag="ps")
                for dkc in range(2):
                    nc.tensor.matmul(
                        s_ps[:],
                        kt_sb[:, dkc, 128 * kt:128 * (kt + 1)],
                        qt_sb[:, dkc, QW * j:QW * (j + 1)],
                        start=(dkc == 0), stop=(dkc == 1))
                p_raw = p_pool.tile([128, QW], bf16, tag="praw")
                nc.scalar.activation(
                    p_raw[:], s_ps[:],
                    mybir.ActivationFunctionType.Exp, scale=SCALE)
                if kt >= E - 8:
                    # only the last 8 tiles of a slot can be causal-cut or
                    # padding (verified for both core groups); others are
                    # provably full -> skip the mask multiply
                    p_m = p_pool.tile([128, QW], bf16, tag="pm")
                    nc.vector.scalar_tensor_tensor(
                        p_m[:], colio_sb[:],
                        thr_sb[:, NKT * j + kt:NKT * j + kt + 1],
                        p_raw[:], AL.is_ge, AL.mult)
                else:
                    p_m = p_raw
                for dvc in range(2):
                    nc.tensor.matmul(
                        o_ps[:, dvc, :],
                        vt_sb[:, kt, 128 * dvc:128 * (dvc + 1)],
                        p_m[:],
                        start=(kt == 0), stop=(kt == E - 1))
                # l: sum 4 P tiles on DVE, one ones-matmul per quad (4x
                # less PE than per-tile lsum; bf16 accum err ~0.2%, fine)
                if kt % 4 == 0:
                    p_acc = p_pool.tile([128, QW], bf16, tag="pacc")
                    nc.vector.tensor_copy(p_acc[:], p_m[:])
                else:
                    nc.vector.tensor_tensor(p_acc[:], p_acc[:], p_m[:], AL.add)
                if kt % 4 == 3:
                    nc.tensor.matmul(
                        l_ps[:], ones_sb[:], p_acc[:],
                        start=(kt == 3), stop=(kt == E - 1))
            o_sb = p_pool.tile([128, 2, QW], f32, tag="osb")
            l_sb = p_pool.tile([1, QW], f32, tag="lsb")
            nc.vector.tensor_copy(o_sb[:], o_ps[:])
            nc.vector.tensor_copy(l_sb[:], l_ps[:])
            for dvc in range(2):
                nc.sync.dma_start(
                    o_t[j, 128 * dvc:128 * (dvc + 1), :], o_sb[:, dvc, :])
            nc.sync.dma_start(l_o[j], l_sb[:])

        # interleave: after col-block cb is projected, slot 3-cb (extent
        # 8*(cb+1)) has exactly the K/V tiles it needs. K/V of cb0 goes
        # first (xt cb0 is the first DMA to land); Q^T follows while the
        # rest of xq streams in.
        proj_kv_block(0)
        proj_q()
        attn_slot(3)
        for cb in range(1, 4):
            proj_kv_block(cb)
            attn_slot(3 - cb)

    nc.compile()
    return nc


def _prep_inputs(inputs, Wq, bq, Wk, bk, Wv, bv):
    import ml_dtypes
    bf16 = ml_dtypes.bfloat16
    in_maps = []
    xt_b = [np.ascontiguousarray(inputs[b].T).astype(bf16) for b in range(B)]
    wq_ = Wq.astype(bf16)
    wk_ = Wk.astype(bf16)
    wv_ = Wv.astype(bf16)
    bqk = np.stack([bq[:128], bq[128:], bk[:128], bk[128:]],
                   axis=1).astype(np.float32)
    bvb = np.tile(bv[None, :], (128, 1)).astype(np.float32)
    colio = np.tile(np.arange(QW, dtype=np.float32)[None, :], (128, 1))
    for core in range(8):
        b, h = core % B, core // B
        chunks = CHUNKS_H[h]
        q0s = [QW * c for c in chunks]
        xq = np.concatenate([xt_b[b][:, q0:q0 + QW] for q0 in q0s], axis=1)
        thr_np = np.empty((128, NSLOT * NKT), dtype=np.float32)
        r = np.arange(128, dtype=np.float32)
        for j in range(NSLOT):
            for kt in range(NKT):
                thr_np[:, NKT * j + kt] = 128 * kt + r - q0s[j]
        cst = np.concatenate([bqk, bvb, colio, thr_np], axis=1)
        in_maps.append({
            "xt": xt_b[b], "xq": np.ascontiguousarray(xq),
            "wq": wq_, "wk": wk_, "wv": wv_,
            "cst": np.ascontiguousarray(cst),
        })
    return in_maps


def kernel(inputs, Wq, bq, Wk, bk, Wv, bv):
    from concourse.bass_utils import run_bass_kernel_spmd

    if "nc" not in _CACHE:
        _CACHE["nc"] = _build_graph()
    nc = _CACHE["nc"]

    in_maps = _prep_inputs(
        np.asarray(inputs), np.asarray(Wq), np.asarray(bq), np.asarray(Wk),
        np.asarray(bk), np.asarray(Wv), np.asarray(bv))

    res = run_bass_kernel_spmd(nc, in_maps, core_ids=list(range(8)))
    _CACHE["last_results"] = res

    out = np.empty((B, T, DK), dtype=np.float32)
    for core in range(8):
        b, h = core % B, core // B
        r = res.results[core]
        o_t, l_v = np.asarray(r["o_t"]), np.asarray(r["l_o"])
        for j, c in enumerate(CHUNKS_H[h]):
            q0 = QW * c
            out[b, q0:q0 + QW, :] = (o_t[j] / l_v[j]).T
    return out


if __name__ == "__main__":
    import reference
    ins = {k: np.asarray(v) for k, v in reference.setup_inputs().items()}
    exp = np.asarray(reference.reference(**{k: v for k, v in ins.items()}))
    act = kernel(**ins)
    err = np.linalg.norm(act - exp) / np.linalg.norm(exp)
    print("Relative error:", err)
